# revision 30
# baseline (speedup 1.0000x reference)
"""Trainium2 Bass kernel for DomainInvariantFeaturesLearningNetwork.

Computation (reference):
  di  = relu(BN(relu(BN(features @ W1)) @ W2))            # [N, H] node feats
  hi  = di @ We1[:H];  hj = di @ We1[H:]                  # edge-net split GEMMs
  logits[i,j] = relu(hi[i] + hj[j] + bwe1) . we2 + bwe2   # all-pairs edge MLP
  w = where(same_label & offdiag, sigmoid(logits), 0)
  out = di + where(wsum>0, (w @ di) / wsum, 0)

Structure: the same_label mask makes the [N,N] edge matrix block-diagonal
after grouping nodes by label.  The host pairs the 16 label groups into 8
(big, small) pairs, one pair per core; the device program has two static
group blocks of sizes (B1, B2) = elementwise max over the pairs (so the
program is SPMD-uniform; per-core group membership arrives as a one-hot
gather matrix + a mask).  Queries and keys of a block share the same slot
window, so the edge work per core is 2*(B1^2 + B2^2) pair columns instead
of the 4x-padded 2*128*256 of a fixed-128 layout.

The MLP runs replicated on every core in transposed space ([H, N] layout,
BN stats along the free dim, pre-BN biases cancel under BN).  Everything
wide is bf16: features (halves the HBM stream), W1/W2/We1, the gather
matrix (one-hot, exact in bf16), and the pair tiles.  Pair tiles
relu(hjT + hi_s + bwe1) are produced per (slot, h-chunk) and spread over
the DVE, Activation and Pool engines in proportion to their modeled
per-tile cost; TensorE reduces each against we2 as a stationary operand
(free ldweights, 1-column moving) into a [keys, slots] logit column.
Junk matmul trains keep the PE pstate at full clock through the BN
stalls.
"""

import numpy as np
import ml_dtypes

import concourse.bass as bass
import concourse.tile as tile
from concourse import mybir
from concourse.bass_utils import run_bass_kernel_spmd

FP32 = mybir.dt.float32
F32R = mybir.dt.float32r
BF16 = mybir.dt.bfloat16
AF = mybir.ActivationFunctionType
OP = mybir.AluOpType

N = 1024          # nodes
FD = 2048         # feature dim
H = 256           # hidden dim (2 partition chunks)
NCORES = 8
P = 128
NG = 16           # label groups
BN_EPS = 1e-5
PAIR_BUFS = 12

_CACHE = {}


def _patch_drain():
    """walrus in this container rejects >1 sync wait on a CTRL instruction;
    split the tile-exit drain waits across sync NOPs, one wait each."""
    if getattr(tile.TileContext, "_drain_patched", False):
        return
    from concourse.tile import ScopedClock

    def _patched(self, tick_clock, wait_clock):
        nop0 = self.nc.sync.nop(nofuse=True, hint="pre_drain_waits")
        wait_clock.add_sem_waits(
            nop0.ins, ScopedClock({None: tick_clock.global_clock})
        )
        si = nop0.ins.sync_info
        if si and si.on_wait and len(si.on_wait) > 1:
            waits = list(si.on_wait)
            si.on_wait = waits[:1]
            for i in range(1, len(waits)):
                nk = self.nc.sync.nop(nofuse=True, hint=f"pre_drain_w{i}")
                nsi = nk.ins.sync_info
                if nsi is None:
                    nk.ins.sync_info = mybir.SyncInfo(
                        on_wait=waits[i : i + 1], on_update=[]
                    )
                else:
                    nsi.on_wait = waits[i : i + 1]
        self.nc.sync.drain()
        self.nc.all_engine_barrier()
        assert self.sems is not None
        popped = self.nc._tile_sem_poison_stack.pop()
        assert popped is self._sem_poison
        self.nc.clear_and_free_semaphores(list(self.sems.allocated().values()))
        self.nc.all_engine_barrier()

    tile.TileContext._drain_and_barrier = _patched
    tile.TileContext._drain_patched = True


def _split_multi_waits(nc):
    """walrus here accepts at most one sync-wait per instruction; hoist
    extras onto same-engine NOPs inserted immediately before (and before
    any contiguous LDWEIGHTS run, so the weight load can't slip past)."""
    idx = 0
    for bb in nc.main_func.blocks:
        new_insts = []
        changed = False
        for ins in bb.instructions:
            si = ins.sync_info
            if si is not None and si.on_wait and len(si.on_wait) > 1:
                waits = list(si.on_wait)
                ip = len(new_insts)
                while (
                    ip > 0
                    and isinstance(new_insts[ip - 1], mybir.InstLdweights)
                    and new_insts[ip - 1].engine == ins.engine
                ):
                    ip -= 1
                for w in waits[:-1]:
                    idx += 1
                    nop = mybir.InstNoOp(
                        name=f"waitsplit_{idx}",
                        engine=ins.engine,
                        sync_info=mybir.SyncInfo(on_wait=[w], on_update=[]),
                        bass_nofuse=True,
                    )
                    nc.register_instruction(nop)
                    new_insts.insert(ip, nop)
                    ip += 1
                si.on_wait = waits[-1:]
                changed = True
            new_insts.append(ins)
        if changed:
            bb.instructions = new_insts
    return nc


def _bn_prep(nc, small, stats, g_col, bt_col, eps_t, ht, sfx=None):
    """From accumulated bn_stats tiles -> (scale, shift) columns for the
    activation-based BN+relu apply."""
    if sfx is None:
        sfx = ht
    mv = small.tile([P, 2], FP32, tag="bn_mv", name=f"mv{sfx}")
    nc.vector.bn_aggr(mv, stats)
    sd = small.tile([P, 1], FP32, tag="bn_sd", name=f"sd{sfx}")
    nc.scalar.activation(sd, mv[:, 1:2], AF.Sqrt, bias=eps_t[:])
    rinv = small.tile([P, 1], FP32, tag="bn_rinv", name=f"ri{sfx}")
    nc.vector.reciprocal(rinv, sd)
    scale = small.tile([P, 1], FP32, tag="bn_scale", name=f"sc{sfx}")
    nc.vector.tensor_mul(scale, rinv, g_col[:, ht : ht + 1])
    ms = small.tile([P, 1], FP32, tag="bn_ms", name=f"ms{sfx}")
    nc.vector.tensor_mul(ms, mv[:, 0:1], scale)
    shift = small.tile([P, 1], FP32, tag="bn_shift", name=f"sh{sfx}")
    nc.vector.tensor_sub(shift, bt_col[:, ht : ht + 1], ms)
    return scale, shift


def _emit_block_epilogue(nc, tc, persist, small, st, b, off, W):
    """sigmoid -> masked bf16 weights -> row-sums -> normalized aggregate
    -> residual add, for one group block; emitted right after the block's
    last logit column so it overlaps the other block's pair production."""
    psum_T = st["psum_T"]
    ep_ps = st["ep_ps"]
    wfin = persist.tile([P, W], FP32, tag=f"wfin{b}", name=f"wfin{b}")
    nc.scalar.activation(
        wfin[:], psum_T[:, off : off + W], AF.Sigmoid, bias=st["bwe2_col"][:]
    )
    wmask = persist.tile([P, W], BF16, tag=f"wmask{b}", name=f"wmask{b}")
    nc.vector.tensor_tensor(
        out=wmask[:], in0=wfin[:], in1=st["mask_sb"][:, off : off + W],
        op=OP.mult,
    )
    p_wsum = ep_ps.tile([P, 1], FP32, tag=f"wsum{b}", name=f"pws{b}", bufs=1)
    nc.tensor.matmul(p_wsum[0:W, :], wmask[:], st["ones_sb"][:],
                     start=True, stop=True)
    rden = small.tile([P, 1], FP32, tag=f"rden{b}", name=f"rden{b}")
    nc.vector.tensor_scalar(out=rden[0:W, :], in0=p_wsum[0:W, :],
                            scalar1=1e-30, scalar2=None, op0=OP.max)
    nc.vector.reciprocal(rden[0:W, :], rden[0:W, :])
    dkn = st["di_keys_nat"][b]
    p_upd = ep_ps.tile([P, H], FP32, tag=f"upd{b}", name=f"pupd{b}", bufs=1)
    nc.tensor.matmul(p_upd[0:W, :], wmask[0:W, :], dkn[0:W, :],
                     start=True, stop=True)
    out_sb = persist.tile([P, H], FP32, tag=f"out_sb{b}", name=f"osb{b}")
    nc.vector.scalar_tensor_tensor(
        out=out_sb[0:W, :], in0=p_upd[0:W, :], scalar=rden[0:W, 0:1],
        in1=dkn[0:W, :], op0=OP.mult, op1=OP.add,
    )
    st["out"][b] = (out_sb, W)
    return st


def _build_program(B1, B2):
    _patch_drain()
    nc = bass.Bass()
    QS = B1 + B2

    featT = nc.declare_dram_parameter("featT", [FD, N], BF16, isOutput=False)
    W1 = nc.declare_dram_parameter("W1", [FD, H], BF16, isOutput=False)
    W2 = nc.declare_dram_parameter("W2", [H, H], BF16, isOutput=False)
    We1a = nc.declare_dram_parameter("We1a", [H, H], BF16, isOutput=False)
    We1b = nc.declare_dram_parameter("We1b", [H, H], BF16, isOutput=False)
    we2 = nc.declare_dram_parameter("we2", [H], FP32, isOutput=False)
    bwe1 = nc.declare_dram_parameter("bwe1", [H], FP32, isOutput=False)
    bwe2 = nc.declare_dram_parameter("bwe2", [1], FP32, isOutput=False)
    g1 = nc.declare_dram_parameter("g1", [H], FP32, isOutput=False)
    bt1 = nc.declare_dram_parameter("bt1", [H], FP32, isOutput=False)
    g2 = nc.declare_dram_parameter("g2", [H], FP32, isOutput=False)
    bt2 = nc.declare_dram_parameter("bt2", [H], FP32, isOutput=False)
    keysel = nc.declare_dram_parameter("keysel", [N, QS], BF16, isOutput=False)
    maskq = nc.declare_dram_parameter("maskq", [P, QS], FP32, isOutput=False)
    ident = nc.declare_dram_parameter("ident", [P, P], BF16, isOutput=False)
    out_block = nc.declare_dram_parameter(
        "out_block", [QS, H], FP32, isOutput=True
    )

    from contextlib import ExitStack

    with tile.TileContext(nc) as tc, ExitStack() as ctx:
        const = ctx.enter_context(tc.tile_pool(name="const", bufs=1))
        persist = ctx.enter_context(tc.tile_pool(name="persist", bufs=1))
        small = ctx.enter_context(tc.tile_pool(name="small", bufs=2))

        # ---- PE warm-up: ramp the clock while weights stream in ---------
        junk = const.tile([P, 512], BF16)
        nc.vector.memset(junk[:], 0.0)
        warm_ps = ctx.enter_context(
            tc.tile_pool(name="warm_ps", bufs=1, space="PSUM")
        )
        warm = warm_ps.tile([P, 512], FP32, name="warm")

        def keep_warm(n, w=512):
            for _ in range(n):
                nc.tensor.matmul(warm[:, 0:w], junk[:, 0:P], junk[:, 0:w],
                                 start=True, stop=True)

        # The cost model fixes an instruction's clock at enqueue time
        # (<=32 instructions ahead): the first 32 PE instructions after any
        # idle run at the mid pstate at best.  Spend them on junk sized so
        # the queue stays busy past the 3us ramp and until the first
        # feature chunk lands; L1 then runs entirely at full clock.
        keep_warm(7)
        keep_warm(32, 32)

        # ---- W1 first, then the feature stream, then the rest: all on
        # the SP queue so nothing preempts the stream on the DMA engines --
        W1r = const.tile([P, FD // P, H], BF16)
        nc.sync.dma_start(
            out=W1r[:], in_=W1[:].rearrange("(c p) h -> p c h", p=P)
        )
        ftr = [const.tile([P, N], BF16, tag=f"ftr{k}", name=f"ftr{k}")
               for k in range(FD // P)]
        for k in range(FD // P):
            nc.sync.dma_start(
                out=ftr[k][:], in_=featT[k * P : (k + 1) * P, :]
            )
        keysel_r = const.tile([P, N // P, QS], BF16)
        nc.sync.dma_start(
            out=keysel_r[:], in_=keysel[:].rearrange("(c p) s -> p c s", p=P)
        )
        W2r = const.tile([P, H // P, H], BF16)
        nc.sync.dma_start(
            out=W2r[:], in_=W2[:].rearrange("(c p) h -> p c h", p=P)
        )
        We1ar = const.tile([P, H // P, H], BF16)
        nc.sync.dma_start(
            out=We1ar[:], in_=We1a[:].rearrange("(c p) h -> p c h", p=P)
        )
        We1br = const.tile([P, H // P, H], BF16)
        nc.sync.dma_start(
            out=We1br[:], in_=We1b[:].rearrange("(c p) h -> p c h", p=P)
        )
        ident_b = const.tile([P, P], BF16)
        nc.sync.dma_start(out=ident_b[:], in_=ident[:])
        mask_sb = const.tile([P, QS], FP32)
        nc.sync.dma_start(out=mask_sb[:], in_=maskq[:])
        we2_sb = const.tile([P, 2], FP32)
        nc.sync.dma_start(
            out=we2_sb[:], in_=we2[:].rearrange("(c p) -> p c", p=P)
        )
        we2_bf = const.tile([P, 2], BF16)
        nc.vector.tensor_copy(we2_bf[:], we2_sb[:])
        cols = {}
        for name, v in (("g1", g1), ("bt1", bt1), ("g2", g2), ("bt2", bt2),
                        ("bwe1", bwe1)):
            t = const.tile([P, 2], FP32, tag=f"col_{name}", name=f"c_{name}")
            nc.sync.dma_start(
                out=t[:], in_=v[:].rearrange("(c p) -> p c", p=P)
            )
            cols[name] = t
        bwe2_col = const.tile([P, 1], FP32)
        nc.gpsimd.dma_start(
            out=bwe2_col[:],
            in_=bass.AP(tensor=bwe2[:].tensor, offset=0, ap=[[0, P], [1, 1]]),
        )
        eps_t = const.tile([P, 1], FP32)
        nc.vector.memset(eps_t[:], BN_EPS)
        ones_sb = const.tile([P, 1], BF16)
        nc.vector.memset(ones_sb[:], 1.0)

        h1T = [persist.tile([P, N], BF16, tag=f"h1T{t}", name=f"h1T{t}")
               for t in range(2)]

        with tc.tile_pool(name="mlp_ps", bufs=2, space="PSUM") as mlp_ps:
            psum_x = [mlp_ps.tile([P, N], FP32, tag="big",
                                  name=f"psum_x{t}") for t in range(2)]
            st1 = [small.tile([P, 2, 6], FP32, tag=f"st1_{t}",
                              name=f"st1_{t}") for t in range(2)]
            for k in range(FD // P):
                for nh in range(2):
                    for ht in range(2):
                        nc.tensor.matmul(
                            psum_x[ht][:, nh * 512 : (nh + 1) * 512],
                            W1r[:, k, ht * P : (ht + 1) * P],
                            ftr[k][:, nh * 512 : (nh + 1) * 512],
                            start=(k == 0),
                            stop=(k == FD // P - 1),
                        )
            scsh1 = []
            for ht in range(2):
                for nh in range(2):
                    nc.vector.bn_stats(
                        st1[ht][:, nh, :],
                        psum_x[ht][:, nh * 512 : (nh + 1) * 512],
                    )
                scsh1.append(_bn_prep(nc, small, st1[ht], cols["g1"],
                                      cols["bt1"], eps_t, ht))
                nc.scalar.activation(
                    h1T[ht][:, 512:1024],
                    psum_x[ht][:, 512:1024],
                    AF.Relu, bias=scsh1[ht][1][:], scale=scsh1[ht][0][:],
                )
            keep_warm(14)
            for ht in range(2):
                scale, shift = scsh1[ht]
                nc.scalar.activation(
                    h1T[ht][:, 0:512],
                    psum_x[ht][:, 0:512],
                    AF.Relu, bias=shift[:], scale=scale[:],
                )

            # ---- L2 transposed: only for the BN2 statistics -------------
            psum_y = [mlp_ps.tile([P, N], FP32, tag="big",
                                  name=f"psum_y{t}") for t in range(2)]
            st2 = [small.tile([P, 2, 6], FP32, tag=f"st2_{t}",
                              name=f"st2_{t}") for t in range(2)]
            x2n = persist.tile([P, N // P, H], BF16, tag="x2n")
            for nh in (1, 0):
                for ht in range(2):
                    for k in range(2):
                        nc.tensor.matmul(
                            psum_y[ht][:, nh * 512 : (nh + 1) * 512],
                            W2r[:, k, ht * P : (ht + 1) * P],
                            h1T[k][:, nh * 512 : (nh + 1) * 512],
                            start=(k == 0),
                            stop=(k == 1),
                        )
                for ht in range(2):
                    nc.vector.bn_stats(
                        st2[ht][:, nh, :],
                        psum_y[ht][:, nh * 512 : (nh + 1) * 512],
                    )
                # natural-layout L2 for this half's node blocks (pre-BN, so
                # only h1T gates it); gather stationary goes via SBUF bf16
                for jp in range(2 * nh, 2 * nh + 2):
                    pn = mlp_ps.tile([P, 2, H], FP32, tag="l2n",
                                     name=f"pn{jp}")
                    for j2 in range(2):
                        jb = jp * 2 + j2
                        for k in range(2):
                            nc.tensor.matmul(
                                pn[:, j2, :],
                                h1T[k][:, jb * P : (jb + 1) * P],
                                W2r[:, k, :],
                                start=(k == 0),
                                stop=(k == 1),
                            )
                    nc.scalar.copy(x2n[:, jp * 2 : jp * 2 + 2, :], pn[:])

        # ---- slot gather + BN2 on just the gathered slots ---------------
        with tc.tile_pool(name="mid_ps", bufs=2, space="PSUM") as mid_ps:

            diT_keys = [
                persist.tile([P, QS], BF16, tag=f"diT_keys{t}",
                             name=f"diT_keys{t}")
                for t in range(2)
            ]
            pxk = [mid_ps.tile([P, QS], FP32, tag="gth", name=f"pxk{t}")
                   for t in range(2)]
            # accumulate in x2n copy-completion order (nh=1 blocks land
            # first), so the gather trails the copies instead of waiting
            # for the last one
            jb_order = list(range(N // P // 2, N // P)) + \
                list(range(N // P // 2))
            for ji, jb in enumerate(jb_order):
                for hc in range(2):
                    nc.tensor.matmul(
                        pxk[hc][:],
                        x2n[:, jb, hc * P : (hc + 1) * P],
                        keysel_r[:, jb, :],
                        start=(ji == 0),
                        stop=(ji == N // P - 1),
                    )
            keep_warm(13)
            for ht in range(2):
                scale, shift = _bn_prep(nc, small, st2[ht], cols["g2"],
                                        cols["bt2"], eps_t, ht, sfx=2 + ht)
                nc.scalar.activation(
                    diT_keys[ht][:], pxk[ht][:], AF.Relu, bias=shift[:],
                    scale=scale[:],
                )

            # hj (bf16) and hi + bwe1 bias columns (f32) for the slots
            hjT_keys = [
                persist.tile([P, QS], BF16, tag=f"hjT_keys{t}",
                             name=f"hjT_keys{t}")
                for t in range(2)
            ]
            bias_all = [
                persist.tile([P, QS], FP32, tag=f"bias_all{t}",
                             name=f"bias_all{t}")
                for t in range(2)
            ]
            for ht in range(2):
                phj = mid_ps.tile([P, QS], FP32, tag="hjp", name=f"phj{ht}")
                for k in range(2):
                    nc.tensor.matmul(
                        phj[:],
                        We1br[:, k, ht * P : (ht + 1) * P],
                        diT_keys[k][:],
                        start=(k == 0),
                        stop=(k == 1),
                    )
                if ht == 0:
                    nc.vector.tensor_copy(hjT_keys[ht][:], phj[:])
                else:
                    nc.scalar.copy(hjT_keys[ht][:], phj[:])
                phi = mid_ps.tile([P, QS], FP32, tag="hjp", name=f"phi{ht}")
                for k in range(2):
                    nc.tensor.matmul(
                        phi[:],
                        We1ar[:, k, ht * P : (ht + 1) * P],
                        diT_keys[k][:],
                        start=(k == 0),
                        stop=(k == 1),
                    )
                nc.vector.tensor_scalar(
                    out=bias_all[ht][:], in0=phi[:],
                    scalar1=cols["bwe1"][:, ht : ht + 1], scalar2=None,
                    op0=OP.add,
                )

            # di keyed naturally per block (queries == keys of the block)
            di_keys_nat = [
                persist.tile([P, H], BF16, tag=f"dkn{b}", name=f"dkn{b}")
                for b in range(2)
            ]
            for b, (off, W) in enumerate(((0, B1), (B1, B2))):
                pst = mid_ps.tile([P, 2, P], BF16, tag="tr", name=f"trk{b}", bufs=1)
                for ht in range(2):
                    nc.tensor.transpose(
                        pst[:W, ht, :], diT_keys[ht][:, off : off + W],
                        ident_b[:],
                    )
                nc.vector.tensor_copy(
                    di_keys_nat[b][0:W, :],
                    pst[0:W, :, :],
                )

        # ---- edge loop --------------------------------------------------
        with (
            tc.tile_pool(name="edge_ps", bufs=1, space="PSUM") as edge_ps,
            tc.tile_pool(name="ep_ps", bufs=2, space="PSUM") as ep_ps,
            tc.tile_pool(name="pair_pool", bufs=1) as pair_pool,
        ):
            psum_T = edge_ps.tile([P, QS], FP32, tag="logitsT")
            nc.vector.memset(psum_T[:], 0.0)
            ep_state = {
                "psum_T": psum_T, "ep_ps": ep_ps, "bwe2_col": bwe2_col,
                "mask_sb": mask_sb, "ones_sb": ones_sb,
                "di_keys_nat": di_keys_nat, "out": {},
            }
            # weighted engine rotation: DVE ~9/16, Act ~3/16, Pool ~4/16
            pattern = [0, 0, 1, 0, 0, 2, 0, 2, 0, 1, 0, 0, 2, 0, 1, 2]
            pi = 0
            for s in range(QS):
                off, W = (0, B1) if s < B1 else (B1, B2)
                pair = [
                    pair_pool.tile([P, W], BF16, tag=f"pr{s}_{t}",
                                   name=f"pair{t}_{s}")
                    for t in range(2)
                ]
                for hc in range(2):
                    e = pattern[pi % len(pattern)]
                    pi += 1
                    if e == 1:
                        nc.scalar.activation(
                            out=pair[hc][:],
                            in_=hjT_keys[hc][:, off : off + W],
                            func=AF.Relu,
                            bias=bias_all[hc][:, s : s + 1],
                        )
                    else:
                        eng = nc.vector if e == 0 else nc.gpsimd
                        eng.tensor_scalar(
                            out=pair[hc][:],
                            in0=hjT_keys[hc][:, off : off + W],
                            scalar1=bias_all[hc][:, s : s + 1],
                            scalar2=0.0,
                            op0=OP.add, op1=OP.max,
                        )
                for hc in range(2):
                    nc.tensor.matmul(
                        psum_T[0:W, s : s + 1],
                        pair[hc][:],
                        we2_bf[:, hc : hc + 1],
                        start=(hc == 0),
                        stop=(hc == 1),
                    )
                if s == B1 - 1:
                    _emit_block_epilogue(
                        nc, tc, persist, small, ep_state, 0, 0, B1)
                elif s == QS - 1:
                    _emit_block_epilogue(
                        nc, tc, persist, small, ep_state, 1, B1, B2)

            for b, (off, W) in enumerate(((0, B1), (B1, B2))):
                out_sb, _ = ep_state["out"][b]
                nc.sync.dma_start(
                    out=out_block[off : off + W, :],
                    in_=out_sb[0:W, :],
                )

    _split_multi_waits(nc)
    return nc


def _get_program(B1=None, B2=None):
    if B1 is None:
        B1, B2 = _CACHE["last_key"]
    key = (B1, B2)
    if key not in _CACHE:
        _CACHE[key] = _build_program(B1, B2)
    return _CACHE[key]


def _host_prep(features, labels, W1, g1, bt1, W2, g2, bt2, We1, bwe1, We2,
               bwe2):
    features = np.asarray(features, dtype=np.float32)
    labels = np.asarray(labels).astype(np.int64)
    We1 = np.asarray(We1, dtype=np.float32)

    counts = np.bincount(labels, minlength=NG)
    if counts.max() > P:
        raise ValueError(f"label group too large: {counts.max()} > {P}")
    order_groups = np.argsort(counts)[::-1]  # groups big -> small
    pairs = [(order_groups[i], order_groups[NG - 1 - i])
             for i in range(NCORES)]
    B1 = int(max(counts[a] for a, _ in pairs))
    B2 = int(max(counts[b] for _, b in pairs))
    B1 = max(B1, 1)
    B2 = max(B2, 1)
    QS = B1 + B2

    by_label = [np.nonzero(labels == v)[0] for v in range(NG)]

    base = {
        "featT": np.ascontiguousarray(features.T).astype(ml_dtypes.bfloat16),
        "W1": np.asarray(W1, np.float32).astype(ml_dtypes.bfloat16),
        "W2": np.asarray(W2, np.float32).astype(ml_dtypes.bfloat16),
        "We1a": np.ascontiguousarray(We1[:H]).astype(ml_dtypes.bfloat16),
        "We1b": np.ascontiguousarray(We1[H:]).astype(ml_dtypes.bfloat16),
        "we2": np.ascontiguousarray(np.asarray(We2, np.float32)[:, 0]),
        "bwe1": np.asarray(bwe1, dtype=np.float32),
        "bwe2": np.asarray(bwe2, dtype=np.float32).reshape(1),
        "g1": np.asarray(g1, dtype=np.float32),
        "bt1": np.asarray(bt1, dtype=np.float32),
        "g2": np.asarray(g2, dtype=np.float32),
        "bt2": np.asarray(bt2, dtype=np.float32),
        "ident": np.eye(P, dtype=np.float32).astype(ml_dtypes.bfloat16),
    }
    in_maps = []
    slot2node = np.full((NCORES, QS), -1, dtype=np.int64)
    for c in range(NCORES):
        ksel = np.zeros((N, QS), dtype=np.float32)
        m = np.zeros((P, QS), dtype=np.float32)
        for b, (off, W) in enumerate(((0, B1), (B1, B2))):
            g = pairs[c][b]
            nodes = by_label[g]
            n = len(nodes)
            slot2node[c, off : off + n] = nodes
            ksel[nodes, off + np.arange(n)] = 1.0
            blk = np.zeros((P, W), dtype=np.float32)
            blk[:n, :n] = 1.0
            np.fill_diagonal(blk, 0.0)
            m[:, off : off + W] = blk
        mm = dict(base)
        mm["keysel"] = ksel.astype(ml_dtypes.bfloat16)
        mm["maskq"] = m
        in_maps.append(mm)
    return in_maps, slot2node, B1, B2


def kernel(features, labels, W1, b1, g1, bt1, W2, b2, g2, bt2,
           We1, bwe1, We2, bwe2, **_unused):
    in_maps, slot2node, B1, B2 = _host_prep(
        features, labels, W1, g1, bt1, W2, g2, bt2, We1, bwe1, We2, bwe2
    )
    nc = _get_program(B1, B2)
    _CACHE["last_in_maps"] = in_maps
    _CACHE["last_key"] = (B1, B2)
    res = run_bass_kernel_spmd(nc, in_maps, list(range(NCORES)))
    _CACHE["last_result"] = res
    out = np.empty((N, H), dtype=np.float32)
    for c in range(NCORES):
        blk = res.results[c]["out_block"]
        slots = slot2node[c]
        real = slots >= 0
        out[slots[real]] = blk[real]
    return out


# revision 31
# speedup vs baseline: 1.0047x; 1.0047x over previous
"""Trainium2 Bass kernel for DomainInvariantFeaturesLearningNetwork.

Computation (reference):
  di  = relu(BN(relu(BN(features @ W1)) @ W2))            # [N, H] node feats
  hi  = di @ We1[:H];  hj = di @ We1[H:]                  # edge-net split GEMMs
  logits[i,j] = relu(hi[i] + hj[j] + bwe1) . we2 + bwe2   # all-pairs edge MLP
  w = where(same_label & offdiag, sigmoid(logits), 0)
  out = di + where(wsum>0, (w @ di) / wsum, 0)

Structure: the same_label mask makes the [N,N] edge matrix block-diagonal
after grouping nodes by label.  The host pairs the 16 label groups into 8
(big, small) pairs, one pair per core; the device program has two static
group blocks of sizes (B1, B2) = elementwise max over the pairs (so the
program is SPMD-uniform; per-core group membership arrives as a one-hot
gather matrix + a mask).  Queries and keys of a block share the same slot
window, so the edge work per core is 2*(B1^2 + B2^2) pair columns instead
of the 4x-padded 2*128*256 of a fixed-128 layout.

The MLP runs replicated on every core in transposed space ([H, N] layout,
BN stats along the free dim, pre-BN biases cancel under BN).  Everything
wide is bf16: features (halves the HBM stream), W1/W2/We1, the gather
matrix (one-hot, exact in bf16), and the pair tiles.  Pair tiles
relu(hjT + hi_s + bwe1) are produced per (slot, h-chunk) and spread over
the DVE, Activation and Pool engines in proportion to their modeled
per-tile cost; TensorE reduces each against we2 as a stationary operand
(free ldweights, 1-column moving) into a [keys, slots] logit column.
Junk matmul trains keep the PE pstate at full clock through the BN
stalls.
"""

import numpy as np
import ml_dtypes

import concourse.bass as bass
import concourse.tile as tile
from concourse import mybir
from concourse.bass_utils import run_bass_kernel_spmd

FP32 = mybir.dt.float32
F32R = mybir.dt.float32r
BF16 = mybir.dt.bfloat16
AF = mybir.ActivationFunctionType
OP = mybir.AluOpType

N = 1024          # nodes
FD = 2048         # feature dim
H = 256           # hidden dim (2 partition chunks)
NCORES = 8
P = 128
NG = 16           # label groups
BN_EPS = 1e-5
PAIR_BUFS = 12

_CACHE = {}


def _patch_drain():
    """walrus in this container rejects >1 sync wait on a CTRL instruction;
    split the tile-exit drain waits across sync NOPs, one wait each."""
    if getattr(tile.TileContext, "_drain_patched", False):
        return
    from concourse.tile import ScopedClock

    def _patched(self, tick_clock, wait_clock):
        nop0 = self.nc.sync.nop(nofuse=True, hint="pre_drain_waits")
        wait_clock.add_sem_waits(
            nop0.ins, ScopedClock({None: tick_clock.global_clock})
        )
        si = nop0.ins.sync_info
        if si and si.on_wait and len(si.on_wait) > 1:
            waits = list(si.on_wait)
            si.on_wait = waits[:1]
            for i in range(1, len(waits)):
                nk = self.nc.sync.nop(nofuse=True, hint=f"pre_drain_w{i}")
                nsi = nk.ins.sync_info
                if nsi is None:
                    nk.ins.sync_info = mybir.SyncInfo(
                        on_wait=waits[i : i + 1], on_update=[]
                    )
                else:
                    nsi.on_wait = waits[i : i + 1]
        self.nc.sync.drain()
        self.nc.all_engine_barrier()
        assert self.sems is not None
        popped = self.nc._tile_sem_poison_stack.pop()
        assert popped is self._sem_poison
        self.nc.clear_and_free_semaphores(list(self.sems.allocated().values()))
        self.nc.all_engine_barrier()

    tile.TileContext._drain_and_barrier = _patched
    tile.TileContext._drain_patched = True


def _split_multi_waits(nc):
    """walrus here accepts at most one sync-wait per instruction; hoist
    extras onto same-engine NOPs inserted immediately before (and before
    any contiguous LDWEIGHTS run, so the weight load can't slip past)."""
    idx = 0
    for bb in nc.main_func.blocks:
        new_insts = []
        changed = False
        for ins in bb.instructions:
            si = ins.sync_info
            if si is not None and si.on_wait and len(si.on_wait) > 1:
                waits = list(si.on_wait)
                ip = len(new_insts)
                while (
                    ip > 0
                    and isinstance(new_insts[ip - 1], mybir.InstLdweights)
                    and new_insts[ip - 1].engine == ins.engine
                ):
                    ip -= 1
                for w in waits[:-1]:
                    idx += 1
                    nop = mybir.InstNoOp(
                        name=f"waitsplit_{idx}",
                        engine=ins.engine,
                        sync_info=mybir.SyncInfo(on_wait=[w], on_update=[]),
                        bass_nofuse=True,
                    )
                    nc.register_instruction(nop)
                    new_insts.insert(ip, nop)
                    ip += 1
                si.on_wait = waits[-1:]
                changed = True
            new_insts.append(ins)
        if changed:
            bb.instructions = new_insts
    return nc


def _bn_prep(nc, small, stats, g_col, bt_col, eps_t, ht, sfx=None):
    """From accumulated bn_stats tiles -> (scale, shift) columns for the
    activation-based BN+relu apply."""
    if sfx is None:
        sfx = ht
    mv = small.tile([P, 2], FP32, tag="bn_mv", name=f"mv{sfx}")
    nc.vector.bn_aggr(mv, stats)
    sd = small.tile([P, 1], FP32, tag="bn_sd", name=f"sd{sfx}")
    nc.scalar.activation(sd, mv[:, 1:2], AF.Sqrt, bias=eps_t[:])
    rinv = small.tile([P, 1], FP32, tag="bn_rinv", name=f"ri{sfx}")
    nc.vector.reciprocal(rinv, sd)
    scale = small.tile([P, 1], FP32, tag="bn_scale", name=f"sc{sfx}")
    nc.vector.tensor_mul(scale, rinv, g_col[:, ht : ht + 1])
    ms = small.tile([P, 1], FP32, tag="bn_ms", name=f"ms{sfx}")
    nc.vector.tensor_mul(ms, mv[:, 0:1], scale)
    shift = small.tile([P, 1], FP32, tag="bn_shift", name=f"sh{sfx}")
    nc.vector.tensor_sub(shift, bt_col[:, ht : ht + 1], ms)
    return scale, shift


def _emit_block_epilogue(nc, tc, persist, small, st, b, off, W):
    """sigmoid -> masked bf16 weights -> row-sums -> normalized aggregate
    -> residual add, for one group block; emitted right after the block's
    last logit column so it overlaps the other block's pair production."""
    psum_T = st["psum_T"]
    ep_ps = st["ep_ps"]
    wfin = persist.tile([P, W], FP32, tag=f"wfin{b}", name=f"wfin{b}")
    nc.scalar.activation(
        wfin[:], psum_T[:, off : off + W], AF.Sigmoid, bias=st["bwe2_col"][:]
    )
    wmask = persist.tile([P, W], BF16, tag=f"wmask{b}", name=f"wmask{b}")
    nc.vector.tensor_tensor(
        out=wmask[:], in0=wfin[:], in1=st["mask_sb"][:, off : off + W],
        op=OP.mult,
    )
    p_wsum = ep_ps.tile([P, 1], FP32, tag=f"wsum{b}", name=f"pws{b}", bufs=1)
    nc.tensor.matmul(p_wsum[0:W, :], wmask[:], st["ones_sb"][:],
                     start=True, stop=True)
    rden = small.tile([P, 1], FP32, tag=f"rden{b}", name=f"rden{b}")
    nc.vector.tensor_scalar(out=rden[0:W, :], in0=p_wsum[0:W, :],
                            scalar1=1e-30, scalar2=None, op0=OP.max)
    nc.vector.reciprocal(rden[0:W, :], rden[0:W, :])
    dkn = st["di_keys_nat"][b]
    p_upd = ep_ps.tile([P, H], FP32, tag=f"upd{b}", name=f"pupd{b}", bufs=1)
    nc.tensor.matmul(p_upd[0:W, :], wmask[0:W, :], dkn[0:W, :],
                     start=True, stop=True)
    out_sb = persist.tile([P, H], FP32, tag=f"out_sb{b}", name=f"osb{b}")
    nc.vector.scalar_tensor_tensor(
        out=out_sb[0:W, :], in0=p_upd[0:W, :], scalar=rden[0:W, 0:1],
        in1=dkn[0:W, :], op0=OP.mult, op1=OP.add,
    )
    st["out"][b] = (out_sb, W)
    return st


def _build_program(B1, B2):
    _patch_drain()
    nc = bass.Bass()
    QS = B1 + B2

    featT = nc.declare_dram_parameter("featT", [FD, N], BF16, isOutput=False)
    W1 = nc.declare_dram_parameter("W1", [FD, H], BF16, isOutput=False)
    W2 = nc.declare_dram_parameter("W2", [H, H], BF16, isOutput=False)
    We1a = nc.declare_dram_parameter("We1a", [H, H], BF16, isOutput=False)
    We1b = nc.declare_dram_parameter("We1b", [H, H], BF16, isOutput=False)
    we2 = nc.declare_dram_parameter("we2", [H], FP32, isOutput=False)
    bwe1 = nc.declare_dram_parameter("bwe1", [H], FP32, isOutput=False)
    bwe2 = nc.declare_dram_parameter("bwe2", [1], FP32, isOutput=False)
    g1 = nc.declare_dram_parameter("g1", [H], FP32, isOutput=False)
    bt1 = nc.declare_dram_parameter("bt1", [H], FP32, isOutput=False)
    g2 = nc.declare_dram_parameter("g2", [H], FP32, isOutput=False)
    bt2 = nc.declare_dram_parameter("bt2", [H], FP32, isOutput=False)
    keysel = nc.declare_dram_parameter("keysel", [N, QS], BF16, isOutput=False)
    maskq = nc.declare_dram_parameter("maskq", [P, QS], FP32, isOutput=False)
    ident = nc.declare_dram_parameter("ident", [P, P], BF16, isOutput=False)
    out_block = nc.declare_dram_parameter(
        "out_block", [QS, H], FP32, isOutput=True
    )

    from contextlib import ExitStack

    with tile.TileContext(nc) as tc, ExitStack() as ctx:
        const = ctx.enter_context(tc.tile_pool(name="const", bufs=1))
        persist = ctx.enter_context(tc.tile_pool(name="persist", bufs=1))
        small = ctx.enter_context(tc.tile_pool(name="small", bufs=2))

        # ---- PE warm-up: ramp the clock while weights stream in ---------
        junk = const.tile([P, 512], BF16)
        nc.vector.memset(junk[:], 0.0)
        warm_ps = ctx.enter_context(
            tc.tile_pool(name="warm_ps", bufs=1, space="PSUM")
        )
        warm = warm_ps.tile([P, 512], FP32, name="warm")

        def keep_warm(n, w=512):
            for _ in range(n):
                nc.tensor.matmul(warm[:, 0:w], junk[:, 0:P], junk[:, 0:w],
                                 start=True, stop=True)

        # The cost model fixes an instruction's clock at enqueue time
        # (<=32 instructions ahead): the first 32 PE instructions after any
        # idle run at the mid pstate at best.  Spend them on junk sized so
        # the queue stays busy past the 3us ramp and until the first
        # feature chunk lands; L1 then runs entirely at full clock.
        keep_warm(7)
        keep_warm(32, 32)

        # ---- W1 first, then the feature stream, then the rest: all on
        # the SP queue so nothing preempts the stream on the DMA engines --
        W1r = const.tile([P, FD // P, H], BF16)
        nc.sync.dma_start(
            out=W1r[:], in_=W1[:].rearrange("(c p) h -> p c h", p=P)
        )
        ftr = [const.tile([P, N], BF16, tag=f"ftr{k}", name=f"ftr{k}")
               for k in range(FD // P)]
        for k in range(FD // P):
            nc.sync.dma_start(
                out=ftr[k][:], in_=featT[k * P : (k + 1) * P, :]
            )
        keysel_r = const.tile([P, N // P, QS], BF16)
        nc.sync.dma_start(
            out=keysel_r[:], in_=keysel[:].rearrange("(c p) s -> p c s", p=P)
        )
        W2r = const.tile([P, H // P, H], BF16)
        nc.sync.dma_start(
            out=W2r[:], in_=W2[:].rearrange("(c p) h -> p c h", p=P)
        )
        We1ar = const.tile([P, H // P, H], BF16)
        nc.sync.dma_start(
            out=We1ar[:], in_=We1a[:].rearrange("(c p) h -> p c h", p=P)
        )
        We1br = const.tile([P, H // P, H], BF16)
        nc.sync.dma_start(
            out=We1br[:], in_=We1b[:].rearrange("(c p) h -> p c h", p=P)
        )
        ident_b = const.tile([P, P], BF16)
        nc.sync.dma_start(out=ident_b[:], in_=ident[:])
        mask_sb = const.tile([P, QS], FP32)
        nc.sync.dma_start(out=mask_sb[:], in_=maskq[:])
        we2_sb = const.tile([P, 2], FP32)
        nc.sync.dma_start(
            out=we2_sb[:], in_=we2[:].rearrange("(c p) -> p c", p=P)
        )
        we2_bf = const.tile([P, 2], BF16)
        nc.vector.tensor_copy(we2_bf[:], we2_sb[:])
        cols = {}
        for name, v in (("g1", g1), ("bt1", bt1), ("g2", g2), ("bt2", bt2),
                        ("bwe1", bwe1)):
            t = const.tile([P, 2], FP32, tag=f"col_{name}", name=f"c_{name}")
            nc.sync.dma_start(
                out=t[:], in_=v[:].rearrange("(c p) -> p c", p=P)
            )
            cols[name] = t
        bwe2_col = const.tile([P, 1], FP32)
        nc.gpsimd.dma_start(
            out=bwe2_col[:],
            in_=bass.AP(tensor=bwe2[:].tensor, offset=0, ap=[[0, P], [1, 1]]),
        )
        eps_t = const.tile([P, 1], FP32)
        nc.vector.memset(eps_t[:], BN_EPS)
        ones_sb = const.tile([P, 1], BF16)
        nc.vector.memset(ones_sb[:], 1.0)

        h1T = [persist.tile([P, N], BF16, tag=f"h1T{t}", name=f"h1T{t}")
               for t in range(2)]

        with tc.tile_pool(name="mlp_ps", bufs=2, space="PSUM") as mlp_ps:
            psum_x = [mlp_ps.tile([P, N], FP32, tag="big",
                                  name=f"psum_x{t}") for t in range(2)]
            st1 = [small.tile([P, 2, 6], FP32, tag=f"st1_{t}",
                              name=f"st1_{t}") for t in range(2)]
            for k in range(FD // P):
                for nh in range(2):
                    for ht in range(2):
                        nc.tensor.matmul(
                            psum_x[ht][:, nh * 512 : (nh + 1) * 512],
                            W1r[:, k, ht * P : (ht + 1) * P],
                            ftr[k][:, nh * 512 : (nh + 1) * 512],
                            start=(k == 0),
                            stop=(k == FD // P - 1),
                        )
            scsh1 = []
            for ht in range(2):
                for nh in range(2):
                    nc.vector.bn_stats(
                        st1[ht][:, nh, :],
                        psum_x[ht][:, nh * 512 : (nh + 1) * 512],
                    )
                scsh1.append(_bn_prep(nc, small, st1[ht], cols["g1"],
                                      cols["bt1"], eps_t, ht))
                nc.scalar.activation(
                    h1T[ht][:, 512:1024],
                    psum_x[ht][:, 512:1024],
                    AF.Relu, bias=scsh1[ht][1][:], scale=scsh1[ht][0][:],
                )
            keep_warm(14)
            for ht in range(2):
                scale, shift = scsh1[ht]
                nc.scalar.activation(
                    h1T[ht][:, 0:512],
                    psum_x[ht][:, 0:512],
                    AF.Relu, bias=shift[:], scale=scale[:],
                )

            # ---- L2 transposed: only for the BN2 statistics -------------
            psum_y = [mlp_ps.tile([P, N], FP32, tag="big",
                                  name=f"psum_y{t}") for t in range(2)]
            st2 = [small.tile([P, 2, 6], FP32, tag=f"st2_{t}",
                              name=f"st2_{t}") for t in range(2)]
            x2n = persist.tile([P, N // P, H], BF16, tag="x2n")
            for nh in (1, 0):
                for ht in range(2):
                    for k in range(2):
                        nc.tensor.matmul(
                            psum_y[ht][:, nh * 512 : (nh + 1) * 512],
                            W2r[:, k, ht * P : (ht + 1) * P],
                            h1T[k][:, nh * 512 : (nh + 1) * 512],
                            start=(k == 0),
                            stop=(k == 1),
                        )
                for ht in range(2):
                    nc.vector.bn_stats(
                        st2[ht][:, nh, :],
                        psum_y[ht][:, nh * 512 : (nh + 1) * 512],
                    )
                # natural-layout L2 for this half's node blocks (pre-BN, so
                # only h1T gates it); gather stationary goes via SBUF bf16
                for jp in range(2 * nh, 2 * nh + 2):
                    pn = mlp_ps.tile([P, 2, H], FP32, tag="l2n",
                                     name=f"pn{jp}")
                    for j2 in range(2):
                        jb = jp * 2 + j2
                        for k in range(2):
                            nc.tensor.matmul(
                                pn[:, j2, :],
                                h1T[k][:, jb * P : (jb + 1) * P],
                                W2r[:, k, :],
                                start=(k == 0),
                                stop=(k == 1),
                            )
                    nc.scalar.copy(x2n[:, jp * 2 : jp * 2 + 2, :], pn[:])

        # ---- slot gather + BN2 on just the gathered slots ---------------
        with tc.tile_pool(name="mid_ps", bufs=2, space="PSUM") as mid_ps:

            diT_keys = [
                persist.tile([P, QS], BF16, tag=f"diT_keys{t}",
                             name=f"diT_keys{t}")
                for t in range(2)
            ]
            pxk = [mid_ps.tile([P, QS], FP32, tag="gth", name=f"pxk{t}")
                   for t in range(2)]
            # accumulate in x2n copy-completion order (nh=1 blocks land
            # first), so the gather trails the copies instead of waiting
            # for the last one
            jb_order = list(range(N // P // 2, N // P)) + \
                list(range(N // P // 2))
            for hc in range(2):
                for ji, jb in enumerate(jb_order):
                    nc.tensor.matmul(
                        pxk[hc][:],
                        x2n[:, jb, hc * P : (hc + 1) * P],
                        keysel_r[:, jb, :],
                        start=(ji == 0),
                        stop=(ji == N // P - 1),
                    )
            keep_warm(13)
            for ht in range(2):
                scale, shift = _bn_prep(nc, small, st2[ht], cols["g2"],
                                        cols["bt2"], eps_t, ht, sfx=2 + ht)
                nc.scalar.activation(
                    diT_keys[ht][:], pxk[ht][:], AF.Relu, bias=shift[:],
                    scale=scale[:],
                )

            # hj (bf16) and hi + bwe1 bias columns (f32) for the slots
            hjT_keys = [
                persist.tile([P, QS], BF16, tag=f"hjT_keys{t}",
                             name=f"hjT_keys{t}")
                for t in range(2)
            ]
            bias_all = [
                persist.tile([P, QS], FP32, tag=f"bias_all{t}",
                             name=f"bias_all{t}")
                for t in range(2)
            ]
            for ht in range(2):
                phj = mid_ps.tile([P, QS], FP32, tag="hjp", name=f"phj{ht}")
                for k in range(2):
                    nc.tensor.matmul(
                        phj[:],
                        We1br[:, k, ht * P : (ht + 1) * P],
                        diT_keys[k][:],
                        start=(k == 0),
                        stop=(k == 1),
                    )
                if ht == 0:
                    nc.vector.tensor_copy(hjT_keys[ht][:], phj[:])
                else:
                    nc.scalar.copy(hjT_keys[ht][:], phj[:])
                phi = mid_ps.tile([P, QS], FP32, tag="hjp", name=f"phi{ht}")
                for k in range(2):
                    nc.tensor.matmul(
                        phi[:],
                        We1ar[:, k, ht * P : (ht + 1) * P],
                        diT_keys[k][:],
                        start=(k == 0),
                        stop=(k == 1),
                    )
                nc.vector.tensor_scalar(
                    out=bias_all[ht][:], in0=phi[:],
                    scalar1=cols["bwe1"][:, ht : ht + 1], scalar2=None,
                    op0=OP.add,
                )

            # di keyed naturally per block (queries == keys of the block)
            di_keys_nat = [
                persist.tile([P, H], BF16, tag=f"dkn{b}", name=f"dkn{b}")
                for b in range(2)
            ]
            for b, (off, W) in enumerate(((0, B1), (B1, B2))):
                pst = mid_ps.tile([P, 2, P], BF16, tag="tr", name=f"trk{b}", bufs=1)
                for ht in range(2):
                    nc.tensor.transpose(
                        pst[:W, ht, :], diT_keys[ht][:, off : off + W],
                        ident_b[:],
                    )
                nc.vector.tensor_copy(
                    di_keys_nat[b][0:W, :],
                    pst[0:W, :, :],
                )

        # ---- edge loop --------------------------------------------------
        with (
            tc.tile_pool(name="edge_ps", bufs=1, space="PSUM") as edge_ps,
            tc.tile_pool(name="ep_ps", bufs=2, space="PSUM") as ep_ps,
            tc.tile_pool(name="pair_pool", bufs=1) as pair_pool,
        ):
            psum_T = edge_ps.tile([P, QS], FP32, tag="logitsT")
            nc.vector.memset(psum_T[:], 0.0)
            ep_state = {
                "psum_T": psum_T, "ep_ps": ep_ps, "bwe2_col": bwe2_col,
                "mask_sb": mask_sb, "ones_sb": ones_sb,
                "di_keys_nat": di_keys_nat, "out": {},
            }
            # weighted engine rotation: DVE ~9/16, Act ~3/16, Pool ~4/16
            pattern = [0, 0, 1, 0, 0, 2, 0, 2, 0, 1, 0, 0, 2, 0, 1, 2]
            pi = 0
            for s in range(QS):
                off, W = (0, B1) if s < B1 else (B1, B2)
                pair = [
                    pair_pool.tile([P, W], BF16, tag=f"pr{s}_{t}",
                                   name=f"pair{t}_{s}")
                    for t in range(2)
                ]
                for hc in range(2):
                    e = pattern[pi % len(pattern)]
                    pi += 1
                    if e == 1:
                        nc.scalar.activation(
                            out=pair[hc][:],
                            in_=hjT_keys[hc][:, off : off + W],
                            func=AF.Relu,
                            bias=bias_all[hc][:, s : s + 1],
                        )
                    else:
                        eng = nc.vector if e == 0 else nc.gpsimd
                        eng.tensor_scalar(
                            out=pair[hc][:],
                            in0=hjT_keys[hc][:, off : off + W],
                            scalar1=bias_all[hc][:, s : s + 1],
                            scalar2=0.0,
                            op0=OP.add, op1=OP.max,
                        )
                for hc in range(2):
                    nc.tensor.matmul(
                        psum_T[0:W, s : s + 1],
                        pair[hc][:],
                        we2_bf[:, hc : hc + 1],
                        start=(hc == 0),
                        stop=(hc == 1),
                    )
                if s == B1 - 1:
                    _emit_block_epilogue(
                        nc, tc, persist, small, ep_state, 0, 0, B1)
                elif s == QS - 1:
                    _emit_block_epilogue(
                        nc, tc, persist, small, ep_state, 1, B1, B2)

            for b, (off, W) in enumerate(((0, B1), (B1, B2))):
                out_sb, _ = ep_state["out"][b]
                nc.sync.dma_start(
                    out=out_block[off : off + W, :],
                    in_=out_sb[0:W, :],
                )

    _split_multi_waits(nc)
    return nc


def _get_program(B1=None, B2=None):
    if B1 is None:
        B1, B2 = _CACHE["last_key"]
    key = (B1, B2)
    if key not in _CACHE:
        _CACHE[key] = _build_program(B1, B2)
    return _CACHE[key]


def _host_prep(features, labels, W1, g1, bt1, W2, g2, bt2, We1, bwe1, We2,
               bwe2):
    features = np.asarray(features, dtype=np.float32)
    labels = np.asarray(labels).astype(np.int64)
    We1 = np.asarray(We1, dtype=np.float32)

    counts = np.bincount(labels, minlength=NG)
    if counts.max() > P:
        raise ValueError(f"label group too large: {counts.max()} > {P}")
    order_groups = np.argsort(counts)[::-1]  # groups big -> small
    pairs = [(order_groups[i], order_groups[NG - 1 - i])
             for i in range(NCORES)]
    B1 = int(max(counts[a] for a, _ in pairs))
    B2 = int(max(counts[b] for _, b in pairs))
    B1 = max(B1, 1)
    B2 = max(B2, 1)
    QS = B1 + B2

    by_label = [np.nonzero(labels == v)[0] for v in range(NG)]

    base = {
        "featT": np.ascontiguousarray(features.T).astype(ml_dtypes.bfloat16),
        "W1": np.asarray(W1, np.float32).astype(ml_dtypes.bfloat16),
        "W2": np.asarray(W2, np.float32).astype(ml_dtypes.bfloat16),
        "We1a": np.ascontiguousarray(We1[:H]).astype(ml_dtypes.bfloat16),
        "We1b": np.ascontiguousarray(We1[H:]).astype(ml_dtypes.bfloat16),
        "we2": np.ascontiguousarray(np.asarray(We2, np.float32)[:, 0]),
        "bwe1": np.asarray(bwe1, dtype=np.float32),
        "bwe2": np.asarray(bwe2, dtype=np.float32).reshape(1),
        "g1": np.asarray(g1, dtype=np.float32),
        "bt1": np.asarray(bt1, dtype=np.float32),
        "g2": np.asarray(g2, dtype=np.float32),
        "bt2": np.asarray(bt2, dtype=np.float32),
        "ident": np.eye(P, dtype=np.float32).astype(ml_dtypes.bfloat16),
    }
    in_maps = []
    slot2node = np.full((NCORES, QS), -1, dtype=np.int64)
    for c in range(NCORES):
        ksel = np.zeros((N, QS), dtype=np.float32)
        m = np.zeros((P, QS), dtype=np.float32)
        for b, (off, W) in enumerate(((0, B1), (B1, B2))):
            g = pairs[c][b]
            nodes = by_label[g]
            n = len(nodes)
            slot2node[c, off : off + n] = nodes
            ksel[nodes, off + np.arange(n)] = 1.0
            blk = np.zeros((P, W), dtype=np.float32)
            blk[:n, :n] = 1.0
            np.fill_diagonal(blk, 0.0)
            m[:, off : off + W] = blk
        mm = dict(base)
        mm["keysel"] = ksel.astype(ml_dtypes.bfloat16)
        mm["maskq"] = m
        in_maps.append(mm)
    return in_maps, slot2node, B1, B2


def kernel(features, labels, W1, b1, g1, bt1, W2, b2, g2, bt2,
           We1, bwe1, We2, bwe2, **_unused):
    in_maps, slot2node, B1, B2 = _host_prep(
        features, labels, W1, g1, bt1, W2, g2, bt2, We1, bwe1, We2, bwe2
    )
    nc = _get_program(B1, B2)
    _CACHE["last_in_maps"] = in_maps
    _CACHE["last_key"] = (B1, B2)
    res = run_bass_kernel_spmd(nc, in_maps, list(range(NCORES)))
    _CACHE["last_result"] = res
    out = np.empty((N, H), dtype=np.float32)
    for c in range(NCORES):
        blk = res.results[c]["out_block"]
        slots = slot2node[c]
        real = slots >= 0
        out[slots[real]] = blk[real]
    return out


# revision 32
# speedup vs baseline: 1.0123x; 1.0076x over previous
"""Trainium2 Bass kernel for DomainInvariantFeaturesLearningNetwork.

Computation (reference):
  di  = relu(BN(relu(BN(features @ W1)) @ W2))            # [N, H] node feats
  hi  = di @ We1[:H];  hj = di @ We1[H:]                  # edge-net split GEMMs
  logits[i,j] = relu(hi[i] + hj[j] + bwe1) . we2 + bwe2   # all-pairs edge MLP
  w = where(same_label & offdiag, sigmoid(logits), 0)
  out = di + where(wsum>0, (w @ di) / wsum, 0)

Structure: the same_label mask makes the [N,N] edge matrix block-diagonal
after grouping nodes by label.  The host pairs the 16 label groups into 8
(big, small) pairs, one pair per core; the device program has two static
group blocks of sizes (B1, B2) = elementwise max over the pairs (so the
program is SPMD-uniform; per-core group membership arrives as a one-hot
gather matrix + a mask).  Queries and keys of a block share the same slot
window, so the edge work per core is 2*(B1^2 + B2^2) pair columns instead
of the 4x-padded 2*128*256 of a fixed-128 layout.

The MLP runs replicated on every core in transposed space ([H, N] layout,
BN stats along the free dim, pre-BN biases cancel under BN).  Everything
wide is bf16: features (halves the HBM stream), W1/W2/We1, the gather
matrix (one-hot, exact in bf16), and the pair tiles.  Pair tiles
relu(hjT + hi_s + bwe1) are produced per (slot, h-chunk) and spread over
the DVE, Activation and Pool engines in proportion to their modeled
per-tile cost; TensorE reduces each against we2 as a stationary operand
(free ldweights, 1-column moving) into a [keys, slots] logit column.
Junk matmul trains keep the PE pstate at full clock through the BN
stalls.
"""

import numpy as np
import ml_dtypes

import concourse.bass as bass
import concourse.tile as tile
from concourse import mybir
from concourse.bass_utils import run_bass_kernel_spmd

FP32 = mybir.dt.float32
F32R = mybir.dt.float32r
BF16 = mybir.dt.bfloat16
AF = mybir.ActivationFunctionType
OP = mybir.AluOpType

N = 1024          # nodes
FD = 2048         # feature dim
H = 256           # hidden dim (2 partition chunks)
NCORES = 8
P = 128
NG = 16           # label groups
BN_EPS = 1e-5
PAIR_BUFS = 12

_CACHE = {}


def _patch_drain():
    """walrus in this container rejects >1 sync wait on a CTRL instruction;
    split the tile-exit drain waits across sync NOPs, one wait each."""
    if getattr(tile.TileContext, "_drain_patched", False):
        return
    from concourse.tile import ScopedClock

    def _patched(self, tick_clock, wait_clock):
        nop0 = self.nc.sync.nop(nofuse=True, hint="pre_drain_waits")
        wait_clock.add_sem_waits(
            nop0.ins, ScopedClock({None: tick_clock.global_clock})
        )
        si = nop0.ins.sync_info
        if si and si.on_wait and len(si.on_wait) > 1:
            waits = list(si.on_wait)
            si.on_wait = waits[:1]
            for i in range(1, len(waits)):
                nk = self.nc.sync.nop(nofuse=True, hint=f"pre_drain_w{i}")
                nsi = nk.ins.sync_info
                if nsi is None:
                    nk.ins.sync_info = mybir.SyncInfo(
                        on_wait=waits[i : i + 1], on_update=[]
                    )
                else:
                    nsi.on_wait = waits[i : i + 1]
        self.nc.sync.drain()
        self.nc.all_engine_barrier()
        assert self.sems is not None
        popped = self.nc._tile_sem_poison_stack.pop()
        assert popped is self._sem_poison
        self.nc.clear_and_free_semaphores(list(self.sems.allocated().values()))
        self.nc.all_engine_barrier()

    tile.TileContext._drain_and_barrier = _patched
    tile.TileContext._drain_patched = True


def _split_multi_waits(nc):
    """walrus here accepts at most one sync-wait per instruction; hoist
    extras onto same-engine NOPs inserted immediately before (and before
    any contiguous LDWEIGHTS run, so the weight load can't slip past)."""
    idx = 0
    for bb in nc.main_func.blocks:
        new_insts = []
        changed = False
        for ins in bb.instructions:
            si = ins.sync_info
            if si is not None and si.on_wait and len(si.on_wait) > 1:
                waits = list(si.on_wait)
                ip = len(new_insts)
                while (
                    ip > 0
                    and isinstance(new_insts[ip - 1], mybir.InstLdweights)
                    and new_insts[ip - 1].engine == ins.engine
                ):
                    ip -= 1
                for w in waits[:-1]:
                    idx += 1
                    nop = mybir.InstNoOp(
                        name=f"waitsplit_{idx}",
                        engine=ins.engine,
                        sync_info=mybir.SyncInfo(on_wait=[w], on_update=[]),
                        bass_nofuse=True,
                    )
                    nc.register_instruction(nop)
                    new_insts.insert(ip, nop)
                    ip += 1
                si.on_wait = waits[-1:]
                changed = True
            new_insts.append(ins)
        if changed:
            bb.instructions = new_insts
    return nc


def _bn_prep(nc, small, stats, g_col, bt_col, eps_t, ht, sfx=None):
    """From accumulated bn_stats tiles -> (scale, shift) columns for the
    activation-based BN+relu apply."""
    if sfx is None:
        sfx = ht
    mv = small.tile([P, 2], FP32, tag="bn_mv", name=f"mv{sfx}")
    nc.vector.bn_aggr(mv, stats)
    sd = small.tile([P, 1], FP32, tag="bn_sd", name=f"sd{sfx}")
    nc.scalar.activation(sd, mv[:, 1:2], AF.Sqrt, bias=eps_t[:])
    rinv = small.tile([P, 1], FP32, tag="bn_rinv", name=f"ri{sfx}")
    nc.vector.reciprocal(rinv, sd)
    scale = small.tile([P, 1], FP32, tag="bn_scale", name=f"sc{sfx}")
    nc.vector.tensor_mul(scale, rinv, g_col[:, ht : ht + 1])
    ms = small.tile([P, 1], FP32, tag="bn_ms", name=f"ms{sfx}")
    nc.vector.tensor_mul(ms, mv[:, 0:1], scale)
    shift = small.tile([P, 1], FP32, tag="bn_shift", name=f"sh{sfx}")
    nc.vector.tensor_sub(shift, bt_col[:, ht : ht + 1], ms)
    return scale, shift


def _emit_block_epilogue(nc, tc, persist, small, st, b, off, W):
    """sigmoid -> masked bf16 weights -> row-sums -> normalized aggregate
    -> residual add, for one group block; emitted right after the block's
    last logit column so it overlaps the other block's pair production."""
    psum_T = st["psum_T"]
    ep_ps = st["ep_ps"]
    wfin = persist.tile([P, W], FP32, tag=f"wfin{b}", name=f"wfin{b}")
    nc.scalar.activation(
        wfin[:], psum_T[:, off : off + W], AF.Sigmoid, bias=st["bwe2_col"][:]
    )
    wmask = persist.tile([P, W], BF16, tag=f"wmask{b}", name=f"wmask{b}")
    nc.vector.tensor_tensor(
        out=wmask[:], in0=wfin[:], in1=st["mask_sb"][:, off : off + W],
        op=OP.mult,
    )
    p_wsum = ep_ps.tile([P, 1], FP32, tag=f"wsum{b}", name=f"pws{b}", bufs=1)
    nc.tensor.matmul(p_wsum[0:W, :], wmask[:], st["ones_sb"][:],
                     start=True, stop=True)
    rden = small.tile([P, 1], FP32, tag=f"rden{b}", name=f"rden{b}")
    nc.vector.tensor_scalar(out=rden[0:W, :], in0=p_wsum[0:W, :],
                            scalar1=1e-30, scalar2=None, op0=OP.max)
    nc.vector.reciprocal(rden[0:W, :], rden[0:W, :])
    dkn = st["di_keys_nat"][b]
    p_upd = ep_ps.tile([P, H], FP32, tag=f"upd{b}", name=f"pupd{b}", bufs=1)
    nc.tensor.matmul(p_upd[0:W, :], wmask[0:W, :], dkn[0:W, :],
                     start=True, stop=True)
    out_sb = persist.tile([P, H], FP32, tag=f"out_sb{b}", name=f"osb{b}")
    nc.vector.scalar_tensor_tensor(
        out=out_sb[0:W, :], in0=p_upd[0:W, :], scalar=rden[0:W, 0:1],
        in1=dkn[0:W, :], op0=OP.mult, op1=OP.add,
    )
    st["out"][b] = (out_sb, W)
    return st


def _build_program(B1, B2):
    _patch_drain()
    nc = bass.Bass()
    QS = B1 + B2

    featT = nc.declare_dram_parameter("featT", [FD, N], BF16, isOutput=False)
    W1 = nc.declare_dram_parameter("W1", [FD, H], BF16, isOutput=False)
    W2 = nc.declare_dram_parameter("W2", [H, H], BF16, isOutput=False)
    We1a = nc.declare_dram_parameter("We1a", [H, H], BF16, isOutput=False)
    We1b = nc.declare_dram_parameter("We1b", [H, H], BF16, isOutput=False)
    we2 = nc.declare_dram_parameter("we2", [H], FP32, isOutput=False)
    bwe1 = nc.declare_dram_parameter("bwe1", [H], FP32, isOutput=False)
    bwe2 = nc.declare_dram_parameter("bwe2", [1], FP32, isOutput=False)
    g1 = nc.declare_dram_parameter("g1", [H], FP32, isOutput=False)
    bt1 = nc.declare_dram_parameter("bt1", [H], FP32, isOutput=False)
    g2 = nc.declare_dram_parameter("g2", [H], FP32, isOutput=False)
    bt2 = nc.declare_dram_parameter("bt2", [H], FP32, isOutput=False)
    keysel = nc.declare_dram_parameter("keysel", [N, QS], BF16, isOutput=False)
    maskq = nc.declare_dram_parameter("maskq", [P, QS], FP32, isOutput=False)
    ident = nc.declare_dram_parameter("ident", [P, P], BF16, isOutput=False)
    out_block = nc.declare_dram_parameter(
        "out_block", [QS, H], FP32, isOutput=True
    )

    from contextlib import ExitStack

    with tile.TileContext(nc) as tc, ExitStack() as ctx:
        const = ctx.enter_context(tc.tile_pool(name="const", bufs=1))
        persist = ctx.enter_context(tc.tile_pool(name="persist", bufs=1))
        small = ctx.enter_context(tc.tile_pool(name="small", bufs=2))

        # ---- PE warm-up: ramp the clock while weights stream in ---------
        junk = const.tile([P, 512], BF16)
        nc.vector.memset(junk[:], 0.0)
        warm_ps = ctx.enter_context(
            tc.tile_pool(name="warm_ps", bufs=1, space="PSUM")
        )
        warm = warm_ps.tile([P, 512], FP32, name="warm")

        def keep_warm(n, w=512):
            for _ in range(n):
                nc.tensor.matmul(warm[:, 0:w], junk[:, 0:P], junk[:, 0:w],
                                 start=True, stop=True)

        # The cost model fixes an instruction's clock at enqueue time
        # (<=32 instructions ahead): the first 32 PE instructions after any
        # idle run at the mid pstate at best.  Spend them on junk sized so
        # the queue stays busy past the 3us ramp and until the first
        # feature chunk lands; L1 then runs entirely at full clock.
        keep_warm(7)
        keep_warm(32, 32)

        # ---- W1 first, then the feature stream, then the rest: all on
        # the SP queue so nothing preempts the stream on the DMA engines --
        W1r = const.tile([P, FD // P, H], BF16)
        ftr = [const.tile([P, N], BF16, tag=f"ftr{k}", name=f"ftr{k}")
               for k in range(FD // P)]
        kg = FD // P // 4
        for g in range(4):
            nc.sync.dma_start(
                out=W1r[:, g * kg : (g + 1) * kg, :],
                in_=W1[g * kg * P : (g + 1) * kg * P, :].rearrange(
                    "(c p) h -> p c h", p=P
                ),
            )
            for k in (range(g * 3, (g + 1) * 3) if g < 3
                      else range(9, FD // P)):
                nc.sync.dma_start(
                    out=ftr[k][:], in_=featT[k * P : (k + 1) * P, :]
                )
        keysel_r = const.tile([P, N // P, QS], BF16)
        nc.sync.dma_start(
            out=keysel_r[:], in_=keysel[:].rearrange("(c p) s -> p c s", p=P)
        )
        W2r = const.tile([P, H // P, H], BF16)
        nc.sync.dma_start(
            out=W2r[:], in_=W2[:].rearrange("(c p) h -> p c h", p=P)
        )
        We1ar = const.tile([P, H // P, H], BF16)
        nc.sync.dma_start(
            out=We1ar[:], in_=We1a[:].rearrange("(c p) h -> p c h", p=P)
        )
        We1br = const.tile([P, H // P, H], BF16)
        nc.sync.dma_start(
            out=We1br[:], in_=We1b[:].rearrange("(c p) h -> p c h", p=P)
        )
        ident_b = const.tile([P, P], BF16)
        nc.sync.dma_start(out=ident_b[:], in_=ident[:])
        mask_sb = const.tile([P, QS], FP32)
        nc.sync.dma_start(out=mask_sb[:], in_=maskq[:])
        we2_sb = const.tile([P, 2], FP32)
        nc.sync.dma_start(
            out=we2_sb[:], in_=we2[:].rearrange("(c p) -> p c", p=P)
        )
        we2_bf = const.tile([P, 2], BF16)
        nc.vector.tensor_copy(we2_bf[:], we2_sb[:])
        cols = {}
        for name, v in (("g1", g1), ("bt1", bt1), ("g2", g2), ("bt2", bt2),
                        ("bwe1", bwe1)):
            t = const.tile([P, 2], FP32, tag=f"col_{name}", name=f"c_{name}")
            nc.sync.dma_start(
                out=t[:], in_=v[:].rearrange("(c p) -> p c", p=P)
            )
            cols[name] = t
        bwe2_col = const.tile([P, 1], FP32)
        nc.gpsimd.dma_start(
            out=bwe2_col[:],
            in_=bass.AP(tensor=bwe2[:].tensor, offset=0, ap=[[0, P], [1, 1]]),
        )
        eps_t = const.tile([P, 1], FP32)
        nc.vector.memset(eps_t[:], BN_EPS)
        ones_sb = const.tile([P, 1], BF16)
        nc.vector.memset(ones_sb[:], 1.0)

        h1T = [persist.tile([P, N], BF16, tag=f"h1T{t}", name=f"h1T{t}")
               for t in range(2)]

        with tc.tile_pool(name="mlp_ps", bufs=2, space="PSUM") as mlp_ps:
            psum_x = [mlp_ps.tile([P, N], FP32, tag="big",
                                  name=f"psum_x{t}") for t in range(2)]
            st1 = [small.tile([P, 2, 6], FP32, tag=f"st1_{t}",
                              name=f"st1_{t}") for t in range(2)]
            for k in range(FD // P):
                for nh in range(2):
                    for ht in range(2):
                        nc.tensor.matmul(
                            psum_x[ht][:, nh * 512 : (nh + 1) * 512],
                            W1r[:, k, ht * P : (ht + 1) * P],
                            ftr[k][:, nh * 512 : (nh + 1) * 512],
                            start=(k == 0),
                            stop=(k == FD // P - 1),
                        )
            scsh1 = []
            for ht in range(2):
                for nh in range(2):
                    nc.vector.bn_stats(
                        st1[ht][:, nh, :],
                        psum_x[ht][:, nh * 512 : (nh + 1) * 512],
                    )
                scsh1.append(_bn_prep(nc, small, st1[ht], cols["g1"],
                                      cols["bt1"], eps_t, ht))
                nc.scalar.activation(
                    h1T[ht][:, 512:1024],
                    psum_x[ht][:, 512:1024],
                    AF.Relu, bias=scsh1[ht][1][:], scale=scsh1[ht][0][:],
                )
            keep_warm(14)
            for ht in range(2):
                scale, shift = scsh1[ht]
                nc.scalar.activation(
                    h1T[ht][:, 0:512],
                    psum_x[ht][:, 0:512],
                    AF.Relu, bias=shift[:], scale=scale[:],
                )

            # ---- L2 transposed: only for the BN2 statistics -------------
            psum_y = [mlp_ps.tile([P, N], FP32, tag="big",
                                  name=f"psum_y{t}") for t in range(2)]
            st2 = [small.tile([P, 2, 6], FP32, tag=f"st2_{t}",
                              name=f"st2_{t}") for t in range(2)]
            x2n = persist.tile([P, N // P, H], BF16, tag="x2n")
            for nh in (1, 0):
                for ht in range(2):
                    for k in range(2):
                        nc.tensor.matmul(
                            psum_y[ht][:, nh * 512 : (nh + 1) * 512],
                            W2r[:, k, ht * P : (ht + 1) * P],
                            h1T[k][:, nh * 512 : (nh + 1) * 512],
                            start=(k == 0),
                            stop=(k == 1),
                        )
                for ht in range(2):
                    nc.vector.bn_stats(
                        st2[ht][:, nh, :],
                        psum_y[ht][:, nh * 512 : (nh + 1) * 512],
                    )
                # natural-layout L2 for this half's node blocks (pre-BN, so
                # only h1T gates it); gather stationary goes via SBUF bf16
                for jp in range(2 * nh, 2 * nh + 2):
                    pn = mlp_ps.tile([P, 2, H], FP32, tag="l2n",
                                     name=f"pn{jp}")
                    for j2 in range(2):
                        jb = jp * 2 + j2
                        for k in range(2):
                            nc.tensor.matmul(
                                pn[:, j2, :],
                                h1T[k][:, jb * P : (jb + 1) * P],
                                W2r[:, k, :],
                                start=(k == 0),
                                stop=(k == 1),
                            )
                    nc.scalar.copy(x2n[:, jp * 2 : jp * 2 + 2, :], pn[:])

        # ---- slot gather + BN2 on just the gathered slots ---------------
        with tc.tile_pool(name="mid_ps", bufs=2, space="PSUM") as mid_ps:

            diT_keys = [
                persist.tile([P, QS], BF16, tag=f"diT_keys{t}",
                             name=f"diT_keys{t}")
                for t in range(2)
            ]
            pxk = [mid_ps.tile([P, QS], FP32, tag="gth", name=f"pxk{t}")
                   for t in range(2)]
            # accumulate in x2n copy-completion order (nh=1 blocks land
            # first), so the gather trails the copies instead of waiting
            # for the last one
            jb_order = list(range(N // P // 2, N // P)) + \
                list(range(N // P // 2))
            for hc in range(2):
                for ji, jb in enumerate(jb_order):
                    nc.tensor.matmul(
                        pxk[hc][:],
                        x2n[:, jb, hc * P : (hc + 1) * P],
                        keysel_r[:, jb, :],
                        start=(ji == 0),
                        stop=(ji == N // P - 1),
                    )
            keep_warm(13)
            for ht in range(2):
                scale, shift = _bn_prep(nc, small, st2[ht], cols["g2"],
                                        cols["bt2"], eps_t, ht, sfx=2 + ht)
                nc.scalar.activation(
                    diT_keys[ht][:], pxk[ht][:], AF.Relu, bias=shift[:],
                    scale=scale[:],
                )

            # hj (bf16) and hi + bwe1 bias columns (f32) for the slots
            hjT_keys = [
                persist.tile([P, QS], BF16, tag=f"hjT_keys{t}",
                             name=f"hjT_keys{t}")
                for t in range(2)
            ]
            bias_all = [
                persist.tile([P, QS], FP32, tag=f"bias_all{t}",
                             name=f"bias_all{t}")
                for t in range(2)
            ]
            for ht in range(2):
                phj = mid_ps.tile([P, QS], FP32, tag="hjp", name=f"phj{ht}")
                for k in range(2):
                    nc.tensor.matmul(
                        phj[:],
                        We1br[:, k, ht * P : (ht + 1) * P],
                        diT_keys[k][:],
                        start=(k == 0),
                        stop=(k == 1),
                    )
                if ht == 0:
                    nc.vector.tensor_copy(hjT_keys[ht][:], phj[:])
                else:
                    nc.scalar.copy(hjT_keys[ht][:], phj[:])
                phi = mid_ps.tile([P, QS], FP32, tag="hjp", name=f"phi{ht}")
                for k in range(2):
                    nc.tensor.matmul(
                        phi[:],
                        We1ar[:, k, ht * P : (ht + 1) * P],
                        diT_keys[k][:],
                        start=(k == 0),
                        stop=(k == 1),
                    )
                nc.vector.tensor_scalar(
                    out=bias_all[ht][:], in0=phi[:],
                    scalar1=cols["bwe1"][:, ht : ht + 1], scalar2=None,
                    op0=OP.add,
                )

            # di keyed naturally per block (queries == keys of the block)
            di_keys_nat = [
                persist.tile([P, H], BF16, tag=f"dkn{b}", name=f"dkn{b}")
                for b in range(2)
            ]
            for b, (off, W) in enumerate(((0, B1), (B1, B2))):
                pst = mid_ps.tile([P, 2, P], BF16, tag="tr", name=f"trk{b}", bufs=1)
                for ht in range(2):
                    nc.tensor.transpose(
                        pst[:W, ht, :], diT_keys[ht][:, off : off + W],
                        ident_b[:],
                    )
                nc.vector.tensor_copy(
                    di_keys_nat[b][0:W, :],
                    pst[0:W, :, :],
                )

        # ---- edge loop --------------------------------------------------
        with (
            tc.tile_pool(name="edge_ps", bufs=1, space="PSUM") as edge_ps,
            tc.tile_pool(name="ep_ps", bufs=2, space="PSUM") as ep_ps,
            tc.tile_pool(name="pair_pool", bufs=1) as pair_pool,
        ):
            psum_T = edge_ps.tile([P, QS], FP32, tag="logitsT")
            nc.vector.memset(psum_T[:], 0.0)
            ep_state = {
                "psum_T": psum_T, "ep_ps": ep_ps, "bwe2_col": bwe2_col,
                "mask_sb": mask_sb, "ones_sb": ones_sb,
                "di_keys_nat": di_keys_nat, "out": {},
            }
            # weighted engine rotation: DVE ~9/16, Act ~3/16, Pool ~4/16
            pattern = [0, 0, 1, 0, 0, 2, 0, 2, 0, 1, 0, 0, 2, 0, 1, 2]
            pi = 0
            for s in range(QS):
                off, W = (0, B1) if s < B1 else (B1, B2)
                pair = [
                    pair_pool.tile([P, W], BF16, tag=f"pr{s}_{t}",
                                   name=f"pair{t}_{s}")
                    for t in range(2)
                ]
                for hc in range(2):
                    e = pattern[pi % len(pattern)]
                    pi += 1
                    if e == 1:
                        nc.scalar.activation(
                            out=pair[hc][:],
                            in_=hjT_keys[hc][:, off : off + W],
                            func=AF.Relu,
                            bias=bias_all[hc][:, s : s + 1],
                        )
                    else:
                        eng = nc.vector if e == 0 else nc.gpsimd
                        eng.tensor_scalar(
                            out=pair[hc][:],
                            in0=hjT_keys[hc][:, off : off + W],
                            scalar1=bias_all[hc][:, s : s + 1],
                            scalar2=0.0,
                            op0=OP.add, op1=OP.max,
                        )
                for hc in range(2):
                    nc.tensor.matmul(
                        psum_T[0:W, s : s + 1],
                        pair[hc][:],
                        we2_bf[:, hc : hc + 1],
                        start=(hc == 0),
                        stop=(hc == 1),
                    )
                if s == B1 - 1:
                    _emit_block_epilogue(
                        nc, tc, persist, small, ep_state, 0, 0, B1)
                elif s == QS - 1:
                    _emit_block_epilogue(
                        nc, tc, persist, small, ep_state, 1, B1, B2)

            for b, (off, W) in enumerate(((0, B1), (B1, B2))):
                out_sb, _ = ep_state["out"][b]
                nc.sync.dma_start(
                    out=out_block[off : off + W, :],
                    in_=out_sb[0:W, :],
                )

    _split_multi_waits(nc)
    return nc


def _get_program(B1=None, B2=None):
    if B1 is None:
        B1, B2 = _CACHE["last_key"]
    key = (B1, B2)
    if key not in _CACHE:
        _CACHE[key] = _build_program(B1, B2)
    return _CACHE[key]


def _host_prep(features, labels, W1, g1, bt1, W2, g2, bt2, We1, bwe1, We2,
               bwe2):
    features = np.asarray(features, dtype=np.float32)
    labels = np.asarray(labels).astype(np.int64)
    We1 = np.asarray(We1, dtype=np.float32)

    counts = np.bincount(labels, minlength=NG)
    if counts.max() > P:
        raise ValueError(f"label group too large: {counts.max()} > {P}")
    order_groups = np.argsort(counts)[::-1]  # groups big -> small
    pairs = [(order_groups[i], order_groups[NG - 1 - i])
             for i in range(NCORES)]
    B1 = int(max(counts[a] for a, _ in pairs))
    B2 = int(max(counts[b] for _, b in pairs))
    B1 = max(B1, 1)
    B2 = max(B2, 1)
    QS = B1 + B2

    by_label = [np.nonzero(labels == v)[0] for v in range(NG)]

    base = {
        "featT": np.ascontiguousarray(features.T).astype(ml_dtypes.bfloat16),
        "W1": np.asarray(W1, np.float32).astype(ml_dtypes.bfloat16),
        "W2": np.asarray(W2, np.float32).astype(ml_dtypes.bfloat16),
        "We1a": np.ascontiguousarray(We1[:H]).astype(ml_dtypes.bfloat16),
        "We1b": np.ascontiguousarray(We1[H:]).astype(ml_dtypes.bfloat16),
        "we2": np.ascontiguousarray(np.asarray(We2, np.float32)[:, 0]),
        "bwe1": np.asarray(bwe1, dtype=np.float32),
        "bwe2": np.asarray(bwe2, dtype=np.float32).reshape(1),
        "g1": np.asarray(g1, dtype=np.float32),
        "bt1": np.asarray(bt1, dtype=np.float32),
        "g2": np.asarray(g2, dtype=np.float32),
        "bt2": np.asarray(bt2, dtype=np.float32),
        "ident": np.eye(P, dtype=np.float32).astype(ml_dtypes.bfloat16),
    }
    in_maps = []
    slot2node = np.full((NCORES, QS), -1, dtype=np.int64)
    for c in range(NCORES):
        ksel = np.zeros((N, QS), dtype=np.float32)
        m = np.zeros((P, QS), dtype=np.float32)
        for b, (off, W) in enumerate(((0, B1), (B1, B2))):
            g = pairs[c][b]
            nodes = by_label[g]
            n = len(nodes)
            slot2node[c, off : off + n] = nodes
            ksel[nodes, off + np.arange(n)] = 1.0
            blk = np.zeros((P, W), dtype=np.float32)
            blk[:n, :n] = 1.0
            np.fill_diagonal(blk, 0.0)
            m[:, off : off + W] = blk
        mm = dict(base)
        mm["keysel"] = ksel.astype(ml_dtypes.bfloat16)
        mm["maskq"] = m
        in_maps.append(mm)
    return in_maps, slot2node, B1, B2


def kernel(features, labels, W1, b1, g1, bt1, W2, b2, g2, bt2,
           We1, bwe1, We2, bwe2, **_unused):
    in_maps, slot2node, B1, B2 = _host_prep(
        features, labels, W1, g1, bt1, W2, g2, bt2, We1, bwe1, We2, bwe2
    )
    nc = _get_program(B1, B2)
    _CACHE["last_in_maps"] = in_maps
    _CACHE["last_key"] = (B1, B2)
    res = run_bass_kernel_spmd(nc, in_maps, list(range(NCORES)))
    _CACHE["last_result"] = res
    out = np.empty((N, H), dtype=np.float32)
    for c in range(NCORES):
        blk = res.results[c]["out_block"]
        slots = slot2node[c]
        real = slots >= 0
        out[slots[real]] = blk[real]
    return out
